# revision 42
# baseline (speedup 1.0000x reference)
"""Trainium2 Bass kernel for nn_Attention (B=4, Nq=Nk=1024, D=512, H=8).

Sharding: 8 cores = 4 batches x 2 head-groups (4 heads each).
Core c handles batch b = c // 2, heads [hg*4, hg*4+4) with hg = c % 2.
Host pre-transposes x/x_q and slices the weights (all bf16), so every
device matmul has its contraction dim on SBUF partitions.

Per-core device program (everything bf16 into f32 PSUM):
  warmup: ~145 junk matmuls during the input-DMA window so the PE's HAM
    clock gate reaches 2.4 GHz before the real work starts.
  qT = (x_q[b] @ w_q[:, hg])^T  as two [128, 1024] pair tiles
  kT per head in a zero-padded [128, 1024] tile (head rows at (h%2)*64,
    other 64 rows zero) so logits matmuls contract over K=128, which
    keeps fast-weight-load enabled; the other operand's junk rows hit 0s.
  v  = x[b] @ w_v [1024, 4, 64] bf16
  per head h:
    A/B interleaved (two independent PE->ACT streams):
      A: logits[q,k] (2 MM) -> exp f32 (ACT, x0.125 + row-sum accumulate)
         -> reciprocal + per-partition normalize (DVE) -> DMA attn out
      B: logitsT[k,q] (2 MM) -> expT bf16 (ACT)
    AV: out[q,64] = sum_kj expT_kj^T v_kj, 8 q-tiles accumulated in one
        PSUM bank, one broadcast-multiply normalize (DVE)
  per pair: PE-transpose out [q,128]->[128,q], project with w_p half,
    DMA a partial [1024, 512] (host sums 2 partials per core x 2 cores
    per batch + bias).
"""

import sys

import numpy as np

for _p in ("/opt/trn_rl_repo",):
    if _p not in sys.path:
        sys.path.insert(0, _p)

# Problem constants (hardcoded per contest rules).
B, NQ, NK = 4, 1024, 1024
D = 512          # DIM_Q = DIM_K = OUT_DIM
H = 8
HD = 64          # head dim
SCALE = HD ** -0.5
HPC = 4          # heads per core
DH = HPC * HD    # 256: per-core slice of head dims
P = 128
KO = D // P      # 4 contraction chunks for the input projections
NCORES = 8

_NC_CACHE = {}


def build_nc():
    """Build the (single) SPMD Bass program. Same program on all 8 cores."""
    from contextlib import ExitStack

    import concourse.mybir as mybir
    import concourse.tile as tile
    from concourse import bacc
    from concourse.masks import make_identity

    FP = mybir.dt.float32
    BF = mybir.dt.bfloat16
    FR = mybir.dt.float32r
    Exp = mybir.ActivationFunctionType.Exp

    nc = bacc.Bacc("TRN2")
    xqT = nc.declare_dram_parameter("xqT", [D, NQ], BF, isOutput=False)
    xT = nc.declare_dram_parameter("xT", [D, NK], BF, isOutput=False)
    wq = nc.declare_dram_parameter("wq", [D, DH], BF, isOutput=False)
    wk = nc.declare_dram_parameter("wk", [D, DH], BF, isOutput=False)
    wv = nc.declare_dram_parameter("wv", [D, DH], BF, isOutput=False)
    wp = nc.declare_dram_parameter("wp", [DH, D], BF, isOutput=False)
    attn_o = nc.declare_dram_parameter("attn_o", [HPC, NQ, NK], FP, isOutput=True)
    out_o = [
        nc.declare_dram_parameter(f"out_o{j}", [NQ, D], FP, isOutput=True)
        for j in range(2)
    ]

    with tile.TileContext(nc) as tc:
        with ExitStack() as ctx:
            consts = ctx.enter_context(tc.tile_pool(name="consts", bufs=1))
            persist = ctx.enter_context(tc.tile_pool(name="persist", bufs=1))
            expT_pool = ctx.enter_context(tc.tile_pool(name="expT", bufs=3))
            exA_pool = ctx.enter_context(tc.tile_pool(name="exA", bufs=12))
            outp = ctx.enter_context(tc.tile_pool(name="outp", bufs=2))
            # PSUM budget (8 banks): lg 3x[128,1024]f32 = 6, sm 2x[128,512]f32 = 2
            ps_sm = ctx.enter_context(tc.tile_pool(name="ps_sm", bufs=2, space="PSUM"))
            ps_lg = ctx.enter_context(tc.tile_pool(name="ps_lg", bufs=3, space="PSUM"))

            # ---- load inputs (split so compute starts early, spread queues) ----
            xqT_sb = persist.tile([P, KO, NQ], BF)
            xT_sb = persist.tile([P, KO, NK], BF)
            for ko in range(KO):
                for sh in range(2):
                    nc.sync.dma_start(
                        xqT_sb[:, ko, sh * 512:(sh + 1) * 512],
                        xqT[:].rearrange("(ko p) q -> ko p q", p=P)
                        [ko, :, sh * 512:(sh + 1) * 512])
                    nc.sync.dma_start(
                        xT_sb[:, ko, sh * 512:(sh + 1) * 512],
                        xT[:].rearrange("(ko p) q -> ko p q", p=P)
                        [ko, :, sh * 512:(sh + 1) * 512])
            wq_sb = persist.tile([P, KO, DH], BF)
            nc.sync.dma_start(wq_sb[:], wq[:].rearrange("(ko p) m -> p ko m", p=P))
            wk_sb = persist.tile([P, KO, DH], BF)
            nc.sync.dma_start(wk_sb[:], wk[:].rearrange("(ko p) m -> p ko m", p=P))
            wv_sb = persist.tile([P, KO, DH], BF)
            nc.sync.dma_start(wv_sb[:], wv[:].rearrange("(ko p) m -> p ko m", p=P))
            wp_sb = persist.tile([P, 2, D], BF)
            nc.sync.dma_start(wp_sb[:], wp[:].rearrange("(j p) n -> p j n", p=P))
            ident = consts.tile([P, P], BF)
            make_identity(nc, ident[:])

            # ---- PE warm-up burst during the input-DMA window ----
            # The HAM clock gate keeps the PE at 1.2 GHz until it sees ~3.4us
            # of sustained matmul activity; without this burst every matmul in
            # the kernel runs at half clock. Junk matmuls on the identity tile
            # keep the PE busy from ~7us (identity ready) until the input DMAs
            # land, so the real matmuls start at 2.4 GHz and stay there.
            psw = ps_lg.tile([P, P], FP, tag="lg", name="warm")
            for _ in range(145):
                nc.tensor.matmul(psw[:], ident[:], ident[:], start=True, stop=True)

            # ---- projections ----
            # qT: [dh-part (2x128), seq]. kT: zero-padded per-head layout
            # [128, head, seq] with rows 64-127 = 0, so every logits matmul
            # contracts over K=128 (enables fast weight load); the junk rows
            # of the other operand multiply zeros.
            qT_js = [persist.tile([P, NQ], BF, name=f"qT{j}") for j in range(2)]
            kT_hs = [persist.tile([P, NK], BF, name=f"kT{h}") for h in range(HPC)]
            for h in range(HPC):
                nc.gpsimd.memset(kT_hs[h][:], 0.0)
            def emit_qT(j):
                for qc in range(2):
                    ps = ps_sm.tile([P, 512], FP, tag="sm", name="psq")
                    for ko in range(KO):
                        nc.tensor.matmul(
                            ps[:],
                            wq_sb[:, ko, j * P:(j + 1) * P],
                            xqT_sb[:, ko, qc * 512:(qc + 1) * 512],
                            start=(ko == 0),
                            stop=(ko == KO - 1),
                        )
                    nc.vector.tensor_copy(qT_js[j][:, qc * 512:(qc + 1) * 512], ps[:])

            # Head h lands on partition rows (h%2)*64..+64 (matching where its
            # qT rows live); the other 64 rows stay zero. Odd heads use
            # column-group tiling so the matmul writes partitions 64-127.
            def emit_kT(h):
                p0 = (h % 2) * 64
                for qc in range(2):
                    ps = ps_sm.tile([P, 512], FP, tag="sm", name="psk")
                    for ko in range(KO):
                        nc.tensor.matmul(
                            ps[p0:p0 + 64],
                            wk_sb[:, ko, h * HD:(h + 1) * HD],
                            xT_sb[:, ko, qc * 512:(qc + 1) * 512],
                            start=(ko == 0),
                            stop=(ko == KO - 1),
                        )
                    nc.vector.tensor_copy(
                        kT_hs[h][p0:p0 + 64, qc * 512:(qc + 1) * 512],
                        ps[p0:p0 + 64])

            # ---- v in natural layout [k-part, kj, head, hd], bf16 ----
            v_sb = persist.tile([P, 8, HPC, HD], BF)

            def emit_v():
                for kj in range(8):
                    ps = ps_sm.tile([P, DH], FP, tag="sm", name="psv_in")
                    for ko in range(KO):
                        nc.tensor.matmul(
                            ps[:],
                            xT_sb[:, ko, kj * P:(kj + 1) * P],
                            wv_sb[:, ko, :],
                            start=(ko == 0),
                            stop=(ko == KO - 1),
                        )
                    nc.vector.tensor_copy(
                        v_sb[:, kj].rearrange("p h d -> p (h d)"), ps[:]
                    )

            # ---- per-head attention stages ----
            # Heads 2*hp (partitions 0-63) and 2*hp+1 (partitions 64-127) run
            # concurrently in the PE via row-group tiling (tile_position is
            # auto-derived from the lhsT/rhs base partition).
            sums = consts.tile([P, HPC * 8], FP)   # row sums per (head, q-tile)
            rec = consts.tile([P, HPC * 8], FP)    # reciprocals
            # normalized attn @ v, [q, dh-half] per pair; and its transpose
            out_ns = [persist.tile([P, 8, P], BF, name=f"outn{j}") for j in range(2)]
            outTs = [persist.tile([P, NQ], BF, name=f"outT{j}") for j in range(2)]

            def phase_AB(h, hp):
                """A: logits [q,k] -> exp f32 (+sums) -> norm -> DMA.
                B: logitsT [k,q] -> expT bf16 (feeds AV). Interleaved so ACT
                always has two independent PE streams to drain."""
                qT_f = qT_js[hp][:]                # [128, NQ] (pair rows)
                kT_h = kT_hs[h][:]                 # [128, NK] (other 64 = 0)
                exa_tiles = {}
                expT = expT_pool.tile([P, 8, NQ], BF, tag="expT", name="expT")
                for i in range(8):
                    psa = ps_lg.tile([P, NK], FP, tag="lg", name="psa")
                    for kc in range(2):
                        nc.tensor.matmul(
                            psa[:, kc * 512:(kc + 1) * 512],
                            qT_f[:, i * P:(i + 1) * P],
                            kT_h[:, kc * 512:(kc + 1) * 512],
                            start=True,
                            stop=True,
                        )
                    exa = exA_pool.tile([P, NK], FP, tag="exA", name="exa")
                    si = h * 8 + i
                    nc.scalar.activation(
                        exa[:], psa[:], Exp, scale=SCALE,
                        accum_out=sums[:, si:si + 1],
                    )
                    exa_tiles[i] = exa

                    psb = ps_lg.tile([P, NQ], FP, tag="lg", name="psb")
                    for qc in range(2):
                        nc.tensor.matmul(
                            psb[:, qc * 512:(qc + 1) * 512],
                            kT_h[:, i * P:(i + 1) * P],
                            qT_f[:, qc * 512:(qc + 1) * 512],
                            start=True,
                            stop=True,
                        )
                    nc.scalar.activation(expT[:, i], psb[:], Exp, scale=SCALE)

                    if i in (3, 7):  # recip + normalize in half batches
                        lo, hi = (0, 4) if i == 3 else (4, 8)
                        nc.vector.reciprocal(
                            rec[:, h * 8 + lo:h * 8 + hi],
                            sums[:, h * 8 + lo:h * 8 + hi],
                        )
                        for mj in range(lo, hi):
                            exa = exa_tiles.pop(mj)
                            sj = h * 8 + mj
                            nc.vector.tensor_scalar_mul(
                                exa[:], exa[:], rec[:, sj:sj + 1]
                            )
                            nc.sync.dma_start(
                                attn_o[h, mj * P:(mj + 1) * P, :], exa[:]
                            )
                return expT

            def phase_AV(h, hp, expT):
                """out_u[q,64] = sum_kj expT_kj^T v_kj; normalize with one
                broadcast multiply. All 8 q-tiles share one PSUM bank."""
                psv = ps_sm.tile([P, 8, HD], FP, tag="sm", name="psv")
                for mi in range(8):
                    for kj in range(8):
                        nc.tensor.matmul(
                            psv[:, mi],
                            expT[:, kj, mi * P:(mi + 1) * P],
                            v_sb[:, kj, h],
                            start=(kj == 0),
                            stop=(kj == 7),
                        )
                nc.vector.tensor_tensor(
                    out_ns[hp][:, :, (h % 2) * HD:(h % 2 + 1) * HD],
                    psv[:],
                    rec[:, h * 8:(h + 1) * 8].unsqueeze(-1).to_broadcast(
                        [P, 8, HD]),
                    mybir.AluOpType.mult,
                )

            def phase_out(hp):
                """Transpose the pair's [q, 128] block and project.

                Pair 1 runs at the kernel tail when the logits PSUM pool is
                idle — use its 3 slots there for a deeper transpose/proj
                pipeline; pair 0 (mid-kernel) stays on the small pool so it
                doesn't stall the exp stream."""
                pool, tag = (ps_lg, "lg") if hp == 1 else (ps_sm, "sm")
                for mi in range(8):
                    pst = pool.tile([P, P], BF, tag=tag, name="pst")
                    nc.tensor.transpose(pst[:], out_ns[hp][:, mi, :], ident[:])
                    nc.vector.tensor_copy(outTs[hp][:, mi * P:(mi + 1) * P], pst[:])
                for mi in range(8):
                    ps = pool.tile([P, D], FP, tag=tag, name="pspj")
                    nc.tensor.matmul(
                        ps[:],
                        outTs[hp][:, mi * P:(mi + 1) * P],
                        wp_sb[:, hp, :],
                        start=True,
                        stop=True,
                    )
                    of = outp.tile([P, D], FP, tag="of")
                    nc.vector.tensor_copy(of[:], ps[:])
                    nc.sync.dma_start(out_o[hp][mi * P:(mi + 1) * P, :], of[:])

            emit_qT(0)
            emit_kT(0)
            emit_kT(1)
            emit_qT(1)
            emit_kT(2)
            emit_kT(3)
            emit_v()
            for hp in range(2):
                h0, h1 = 2 * hp, 2 * hp + 1
                ea = phase_AB(h0, hp)
                phase_AV(h0, hp, ea)
                eb = phase_AB(h1, hp)
                phase_AV(h1, hp, eb)
                phase_out(hp)

    nc.compile()
    return nc


def get_nc():
    if "nc" not in _NC_CACHE:
        _NC_CACHE["nc"] = build_nc()
    return _NC_CACHE["nc"]


def make_in_maps(x, x_q, w_q, w_kv):
    """Shard full inputs into 8 per-core input maps (host-side numpy)."""
    import ml_dtypes

    bf = ml_dtypes.bfloat16
    x = np.asarray(x, dtype=np.float32)
    x_q = np.asarray(x_q, dtype=np.float32)
    w_q = np.asarray(w_q, dtype=np.float32)
    w_kv = np.asarray(w_kv, dtype=np.float32)
    xqT_b = [np.ascontiguousarray(x_q[b].T.astype(bf)) for b in range(B)]
    xT_b = [np.ascontiguousarray(x[b].T.astype(bf)) for b in range(B)]
    in_maps = []
    for c in range(NCORES):
        b, hg = c // 2, c % 2
        sl = slice(hg * DH, (hg + 1) * DH)
        in_maps.append({
            "xqT": xqT_b[b],
            "xT": xT_b[b],
            "wq": np.ascontiguousarray(w_q[:, sl].astype(bf)),
            "wk": np.ascontiguousarray(w_kv[:, sl].astype(bf)),
            "wv": np.ascontiguousarray(
                w_kv[:, D + hg * DH:D + (hg + 1) * DH].astype(bf)),
        })
    return in_maps


def make_in_maps_full(x, x_q, w_q, w_kv, w_proj):
    import ml_dtypes

    w_proj = np.asarray(w_proj, dtype=np.float32)
    in_maps = make_in_maps(x, x_q, w_q, w_kv)
    for c in range(NCORES):
        hg = c % 2
        sl = slice(hg * DH, (hg + 1) * DH)
        in_maps[c]["wp"] = np.ascontiguousarray(
            w_proj[sl, :].astype(ml_dtypes.bfloat16)
        )
    return in_maps


def unshard(results, b_proj):
    b_proj = np.asarray(b_proj, dtype=np.float32)
    attn = np.empty((B, H, NQ, NK), dtype=np.float32)
    out = np.empty((B, NQ, D), dtype=np.float32)
    for c in range(NCORES):
        b, hg = c // 2, c % 2
        attn[b, hg * HPC:(hg + 1) * HPC] = results[c]["attn_o"]
    for b in range(B):
        out[b] = (
            results[2 * b]["out_o0"] + results[2 * b]["out_o1"]
            + results[2 * b + 1]["out_o0"] + results[2 * b + 1]["out_o1"]
            + b_proj[None, :]
        )
    return out, attn


def kernel(x, x_q, w_q, w_kv, w_proj, b_proj):
    from concourse.bass_utils import run_bass_kernel_spmd

    nc = get_nc()
    in_maps = make_in_maps_full(x, x_q, w_q, w_kv, w_proj)
    res = run_bass_kernel_spmd(nc, in_maps, list(range(NCORES))).results
    return unshard(res, b_proj)


# revision 43
# speedup vs baseline: 1.0086x; 1.0086x over previous
"""Trainium2 Bass kernel for nn_Attention (B=4, Nq=Nk=1024, D=512, H=8).

Sharding: 8 cores = 4 batches x 2 head-groups (4 heads each).
Core c handles batch b = c // 2, heads [hg*4, hg*4+4) with hg = c % 2.
Host pre-transposes x/x_q and slices the weights (all bf16), so every
device matmul has its contraction dim on SBUF partitions.

Per-core device program (everything bf16 into f32 PSUM):
  warmup: ~145 junk matmuls during the input-DMA window so the PE's HAM
    clock gate reaches 2.4 GHz before the real work starts.
  qT = (x_q[b] @ w_q[:, hg])^T  as two [128, 1024] pair tiles
  kT per head in a zero-padded [128, 1024] tile (head rows at (h%2)*64,
    other 64 rows zero) so logits matmuls contract over K=128, which
    keeps fast-weight-load enabled; the other operand's junk rows hit 0s.
  v  = x[b] @ w_v [1024, 4, 64] bf16
  per head h:
    A/B interleaved (two independent PE->ACT streams):
      A: logits[q,k] (2 MM) -> exp f32 (ACT, x0.125 + row-sum accumulate)
         -> reciprocal + per-partition normalize (DVE) -> DMA attn out
      B: logitsT[k,q] (2 MM) -> expT bf16 (ACT)
    AV: out[q,64] = sum_kj expT_kj^T v_kj, 8 q-tiles accumulated in one
        PSUM bank, one broadcast-multiply normalize (DVE)
  per pair: PE-transpose out [q,128]->[128,q], project with w_p half,
    DMA a partial [1024, 512] (host sums 2 partials per core x 2 cores
    per batch + bias).
"""

import sys

import numpy as np

for _p in ("/opt/trn_rl_repo",):
    if _p not in sys.path:
        sys.path.insert(0, _p)

# Problem constants (hardcoded per contest rules).
B, NQ, NK = 4, 1024, 1024
D = 512          # DIM_Q = DIM_K = OUT_DIM
H = 8
HD = 64          # head dim
SCALE = HD ** -0.5
HPC = 4          # heads per core
DH = HPC * HD    # 256: per-core slice of head dims
P = 128
KO = D // P      # 4 contraction chunks for the input projections
NCORES = 8

_NC_CACHE = {}


def build_nc():
    """Build the (single) SPMD Bass program. Same program on all 8 cores."""
    from contextlib import ExitStack

    import concourse.mybir as mybir
    import concourse.tile as tile
    from concourse import bacc
    from concourse.masks import make_identity

    FP = mybir.dt.float32
    BF = mybir.dt.bfloat16
    FR = mybir.dt.float32r
    Exp = mybir.ActivationFunctionType.Exp

    nc = bacc.Bacc("TRN2")
    xqT = nc.declare_dram_parameter("xqT", [D, NQ], BF, isOutput=False)
    xT = nc.declare_dram_parameter("xT", [D, NK], BF, isOutput=False)
    wq = nc.declare_dram_parameter("wq", [D, DH], BF, isOutput=False)
    wk = nc.declare_dram_parameter("wk", [D, DH], BF, isOutput=False)
    wv = nc.declare_dram_parameter("wv", [D, DH], BF, isOutput=False)
    wp = nc.declare_dram_parameter("wp", [DH, D], BF, isOutput=False)
    attn_o = nc.declare_dram_parameter("attn_o", [HPC, NQ, NK], FP, isOutput=True)
    out_o = [
        nc.declare_dram_parameter(f"out_o{j}", [NQ, D], FP, isOutput=True)
        for j in range(2)
    ]

    with tile.TileContext(nc) as tc:
        with ExitStack() as ctx:
            consts = ctx.enter_context(tc.tile_pool(name="consts", bufs=1))
            persist = ctx.enter_context(tc.tile_pool(name="persist", bufs=1))
            expT_pool = ctx.enter_context(tc.tile_pool(name="expT", bufs=3))
            exA_pool = ctx.enter_context(tc.tile_pool(name="exA", bufs=12))
            outp = ctx.enter_context(tc.tile_pool(name="outp", bufs=2))
            # PSUM budget (8 banks): lg 3x[128,1024]f32 = 6, sm 2x[128,512]f32 = 2
            ps_sm = ctx.enter_context(tc.tile_pool(name="ps_sm", bufs=2, space="PSUM"))
            ps_lg = ctx.enter_context(tc.tile_pool(name="ps_lg", bufs=3, space="PSUM"))

            # ---- load inputs (split so compute starts early, spread queues) ----
            xqT_sb = persist.tile([P, KO, NQ], BF)
            xT_sb = persist.tile([P, KO, NK], BF)
            for ko in range(KO):
                for sh in range(2):
                    nc.sync.dma_start(
                        xqT_sb[:, ko, sh * 512:(sh + 1) * 512],
                        xqT[:].rearrange("(ko p) q -> ko p q", p=P)
                        [ko, :, sh * 512:(sh + 1) * 512])
                    nc.sync.dma_start(
                        xT_sb[:, ko, sh * 512:(sh + 1) * 512],
                        xT[:].rearrange("(ko p) q -> ko p q", p=P)
                        [ko, :, sh * 512:(sh + 1) * 512])
            wq_sb = persist.tile([P, KO, DH], BF)
            nc.sync.dma_start(wq_sb[:], wq[:].rearrange("(ko p) m -> p ko m", p=P))
            wk_sb = persist.tile([P, KO, DH], BF)
            nc.sync.dma_start(wk_sb[:], wk[:].rearrange("(ko p) m -> p ko m", p=P))
            wv_sb = persist.tile([P, KO, DH], BF)
            nc.sync.dma_start(wv_sb[:], wv[:].rearrange("(ko p) m -> p ko m", p=P))
            wp_sb = persist.tile([P, 2, D], BF)
            nc.sync.dma_start(wp_sb[:], wp[:].rearrange("(j p) n -> p j n", p=P))
            ident = consts.tile([P, P], BF)
            make_identity(nc, ident[:])

            # ---- PE warm-up burst during the input-DMA window ----
            # The HAM clock gate keeps the PE at 1.2 GHz until it sees ~3.4us
            # of sustained matmul activity; without this burst every matmul in
            # the kernel runs at half clock. Junk matmuls on the identity tile
            # keep the PE busy from ~7us (identity ready) until the input DMAs
            # land, so the real matmuls start at 2.4 GHz and stay there.
            psw = ps_lg.tile([P, P], FP, tag="lg", name="warm")
            for _ in range(145):
                nc.tensor.matmul(psw[:], ident[:], ident[:], start=True, stop=True)

            # ---- projections ----
            # qT: [dh-part (2x128), seq]. kT: zero-padded per-head layout
            # [128, head, seq] with rows 64-127 = 0, so every logits matmul
            # contracts over K=128 (enables fast weight load); the junk rows
            # of the other operand multiply zeros.
            qT_js = [persist.tile([P, NQ], BF, name=f"qT{j}") for j in range(2)]
            kT_hs = [persist.tile([P, NK], BF, name=f"kT{h}") for h in range(HPC)]
            for h in range(HPC):
                nc.gpsimd.memset(kT_hs[h][:], 0.0)
            def emit_qT(j, pool=None, tag="sm"):
                pool = pool or ps_sm
                for qc in range(2):
                    ps = pool.tile([P, 512], FP, tag=tag, name="psq")
                    for ko in range(KO):
                        nc.tensor.matmul(
                            ps[:],
                            wq_sb[:, ko, j * P:(j + 1) * P],
                            xqT_sb[:, ko, qc * 512:(qc + 1) * 512],
                            start=(ko == 0),
                            stop=(ko == KO - 1),
                        )
                    nc.vector.tensor_copy(qT_js[j][:, qc * 512:(qc + 1) * 512], ps[:])

            # Head h lands on partition rows (h%2)*64..+64 (matching where its
            # qT rows live); the other 64 rows stay zero. Odd heads use
            # column-group tiling so the matmul writes partitions 64-127.
            def emit_kT(h, pool=None, tag="sm"):
                pool = pool or ps_sm
                p0 = (h % 2) * 64
                for qc in range(2):
                    ps = pool.tile([P, 512], FP, tag=tag, name="psk")
                    for ko in range(KO):
                        nc.tensor.matmul(
                            ps[p0:p0 + 64],
                            wk_sb[:, ko, h * HD:(h + 1) * HD],
                            xT_sb[:, ko, qc * 512:(qc + 1) * 512],
                            start=(ko == 0),
                            stop=(ko == KO - 1),
                        )
                    nc.vector.tensor_copy(
                        kT_hs[h][p0:p0 + 64, qc * 512:(qc + 1) * 512],
                        ps[p0:p0 + 64])

            # ---- v in natural layout [k-part, kj, head, hd], bf16 ----
            v_sb = persist.tile([P, 8, HPC, HD], BF)

            def emit_v():
                for kj in range(8):
                    ps = ps_sm.tile([P, DH], FP, tag="sm", name="psv_in")
                    for ko in range(KO):
                        nc.tensor.matmul(
                            ps[:],
                            xT_sb[:, ko, kj * P:(kj + 1) * P],
                            wv_sb[:, ko, :],
                            start=(ko == 0),
                            stop=(ko == KO - 1),
                        )
                    nc.vector.tensor_copy(
                        v_sb[:, kj].rearrange("p h d -> p (h d)"), ps[:]
                    )

            # ---- per-head attention stages ----
            # Heads 2*hp (partitions 0-63) and 2*hp+1 (partitions 64-127) run
            # concurrently in the PE via row-group tiling (tile_position is
            # auto-derived from the lhsT/rhs base partition).
            sums = consts.tile([P, HPC * 8], FP)   # row sums per (head, q-tile)
            rec = consts.tile([P, HPC * 8], FP)    # reciprocals
            # normalized attn @ v, [q, dh-half] per pair; and its transpose
            out_ns = [persist.tile([P, 8, P], BF, name=f"outn{j}") for j in range(2)]
            outTs = [persist.tile([P, NQ], BF, name=f"outT{j}") for j in range(2)]

            def phase_AB(h, hp):
                """A: logits [q,k] -> exp f32 (+sums) -> norm -> DMA.
                B: logitsT [k,q] -> expT bf16 (feeds AV). Interleaved so ACT
                always has two independent PE streams to drain."""
                qT_f = qT_js[hp][:]                # [128, NQ] (pair rows)
                kT_h = kT_hs[h][:]                 # [128, NK] (other 64 = 0)
                exa_tiles = {}
                expT = expT_pool.tile([P, 8, NQ], BF, tag="expT", name="expT")
                for i in range(8):
                    psa = ps_lg.tile([P, NK], FP, tag="lg", name="psa")
                    for kc in range(2):
                        nc.tensor.matmul(
                            psa[:, kc * 512:(kc + 1) * 512],
                            qT_f[:, i * P:(i + 1) * P],
                            kT_h[:, kc * 512:(kc + 1) * 512],
                            start=True,
                            stop=True,
                        )
                    exa = exA_pool.tile([P, NK], FP, tag="exA", name="exa")
                    si = h * 8 + i
                    nc.scalar.activation(
                        exa[:], psa[:], Exp, scale=SCALE,
                        accum_out=sums[:, si:si + 1],
                    )
                    exa_tiles[i] = exa

                    psb = ps_lg.tile([P, NQ], FP, tag="lg", name="psb")
                    for qc in range(2):
                        nc.tensor.matmul(
                            psb[:, qc * 512:(qc + 1) * 512],
                            kT_h[:, i * P:(i + 1) * P],
                            qT_f[:, qc * 512:(qc + 1) * 512],
                            start=True,
                            stop=True,
                        )
                    nc.scalar.activation(expT[:, i], psb[:], Exp, scale=SCALE)

                    if i in (3, 7):  # recip + normalize in half batches
                        lo, hi = (0, 4) if i == 3 else (4, 8)
                        nc.vector.reciprocal(
                            rec[:, h * 8 + lo:h * 8 + hi],
                            sums[:, h * 8 + lo:h * 8 + hi],
                        )
                        for mj in range(lo, hi):
                            exa = exa_tiles.pop(mj)
                            sj = h * 8 + mj
                            nc.vector.tensor_scalar_mul(
                                exa[:], exa[:], rec[:, sj:sj + 1]
                            )
                            nc.sync.dma_start(
                                attn_o[h, mj * P:(mj + 1) * P, :], exa[:]
                            )
                return expT

            def phase_AV(h, hp, expT):
                """out_u[q,64] = sum_kj expT_kj^T v_kj; normalize with one
                broadcast multiply. All 8 q-tiles share one PSUM bank."""
                psv = ps_sm.tile([P, 8, HD], FP, tag="sm", name="psv")
                for mi in range(8):
                    for kj in range(8):
                        nc.tensor.matmul(
                            psv[:, mi],
                            expT[:, kj, mi * P:(mi + 1) * P],
                            v_sb[:, kj, h],
                            start=(kj == 0),
                            stop=(kj == 7),
                        )
                nc.vector.tensor_tensor(
                    out_ns[hp][:, :, (h % 2) * HD:(h % 2 + 1) * HD],
                    psv[:],
                    rec[:, h * 8:(h + 1) * 8].unsqueeze(-1).to_broadcast(
                        [P, 8, HD]),
                    mybir.AluOpType.mult,
                )

            def phase_out(hp):
                """Transpose the pair's [q, 128] block and project.

                Pair 1 runs at the kernel tail when the logits PSUM pool is
                idle — use its 3 slots there for a deeper transpose/proj
                pipeline; pair 0 (mid-kernel) stays on the small pool so it
                doesn't stall the exp stream."""
                pool, tag = (ps_lg, "lg") if hp == 1 else (ps_sm, "sm")
                for mi in range(8):
                    pst = pool.tile([P, P], BF, tag=tag, name="pst")
                    nc.tensor.transpose(pst[:], out_ns[hp][:, mi, :], ident[:])
                    nc.vector.tensor_copy(outTs[hp][:, mi * P:(mi + 1) * P], pst[:])
                for mi in range(8):
                    ps = pool.tile([P, D], FP, tag=tag, name="pspj")
                    nc.tensor.matmul(
                        ps[:],
                        outTs[hp][:, mi * P:(mi + 1) * P],
                        wp_sb[:, hp, :],
                        start=True,
                        stop=True,
                    )
                    of = outp.tile([P, D], FP, tag="of")
                    nc.vector.tensor_copy(of[:], ps[:])
                    nc.sync.dma_start(out_o[hp][mi * P:(mi + 1) * P, :], of[:])

            emit_qT(0, pool=ps_lg, tag="lg")
            emit_kT(0, pool=ps_lg, tag="lg")
            emit_kT(1)
            emit_qT(1)
            emit_kT(2)
            emit_kT(3)
            emit_v()
            for hp in range(2):
                h0, h1 = 2 * hp, 2 * hp + 1
                ea = phase_AB(h0, hp)
                phase_AV(h0, hp, ea)
                eb = phase_AB(h1, hp)
                phase_AV(h1, hp, eb)
                phase_out(hp)

    nc.compile()
    return nc


def get_nc():
    if "nc" not in _NC_CACHE:
        _NC_CACHE["nc"] = build_nc()
    return _NC_CACHE["nc"]


def make_in_maps(x, x_q, w_q, w_kv):
    """Shard full inputs into 8 per-core input maps (host-side numpy)."""
    import ml_dtypes

    bf = ml_dtypes.bfloat16
    x = np.asarray(x, dtype=np.float32)
    x_q = np.asarray(x_q, dtype=np.float32)
    w_q = np.asarray(w_q, dtype=np.float32)
    w_kv = np.asarray(w_kv, dtype=np.float32)
    xqT_b = [np.ascontiguousarray(x_q[b].T.astype(bf)) for b in range(B)]
    xT_b = [np.ascontiguousarray(x[b].T.astype(bf)) for b in range(B)]
    in_maps = []
    for c in range(NCORES):
        b, hg = c // 2, c % 2
        sl = slice(hg * DH, (hg + 1) * DH)
        in_maps.append({
            "xqT": xqT_b[b],
            "xT": xT_b[b],
            "wq": np.ascontiguousarray(w_q[:, sl].astype(bf)),
            "wk": np.ascontiguousarray(w_kv[:, sl].astype(bf)),
            "wv": np.ascontiguousarray(
                w_kv[:, D + hg * DH:D + (hg + 1) * DH].astype(bf)),
        })
    return in_maps


def make_in_maps_full(x, x_q, w_q, w_kv, w_proj):
    import ml_dtypes

    w_proj = np.asarray(w_proj, dtype=np.float32)
    in_maps = make_in_maps(x, x_q, w_q, w_kv)
    for c in range(NCORES):
        hg = c % 2
        sl = slice(hg * DH, (hg + 1) * DH)
        in_maps[c]["wp"] = np.ascontiguousarray(
            w_proj[sl, :].astype(ml_dtypes.bfloat16)
        )
    return in_maps


def unshard(results, b_proj):
    b_proj = np.asarray(b_proj, dtype=np.float32)
    attn = np.empty((B, H, NQ, NK), dtype=np.float32)
    out = np.empty((B, NQ, D), dtype=np.float32)
    for c in range(NCORES):
        b, hg = c // 2, c % 2
        attn[b, hg * HPC:(hg + 1) * HPC] = results[c]["attn_o"]
    for b in range(B):
        out[b] = (
            results[2 * b]["out_o0"] + results[2 * b]["out_o1"]
            + results[2 * b + 1]["out_o0"] + results[2 * b + 1]["out_o1"]
            + b_proj[None, :]
        )
    return out, attn


def kernel(x, x_q, w_q, w_kv, w_proj, b_proj):
    from concourse.bass_utils import run_bass_kernel_spmd

    nc = get_nc()
    in_maps = make_in_maps_full(x, x_q, w_q, w_kv, w_proj)
    res = run_bass_kernel_spmd(nc, in_maps, list(range(NCORES))).results
    return unshard(res, b_proj)
